# revision 1
# baseline (speedup 1.0000x reference)
"""ConvAttention (linear attention with conv projections) on 8 trn2 cores.

Sharding: data-parallel over batch B=8, one image per NeuronCore.

Per-core pipeline (channel-major activations [chan, tok], tok = y*64+x):
  q      = Wq @ f                 PE, channel-major psum -> exp -> bf16 sbuf
  dw     = depthwise3x3(f)        DVE, 9 fused (tap*w + acc) ops per c-tile
  kv^T   = dw^T @ Wkv^T           PE, token-major psum; k-half -> exp, v-half copy
  Sk     = ones^T @ exp_k         PE row-sum accumulated over token tiles
  ctx    = exp_k^T @ v            PE, 2-head-packed [128,128] blocks
  ctxn   = ctx * (1/Sk) * scale   DVE per-partition scalars, into block-diag tile
  Sq     = blockdiag1 @ exp_q     PE: per-head partition sums broadcast to 128 rows
  eqn    = exp_q * (1/Sq)         DVE reciprocal + multiply
  att    = ctxn_bd^T @ eqn        PE, channel-major
  g      = gelu(att)              ACT, psum -> bf16 sbuf
  out    = Wout @ g + bout        PE (bias via K=1 ones matmul), psum -> DRAM DMA
"""

import numpy as np
import ml_dtypes

B, C, H, W = 8, 256, 64, 64
HEADS, HID = 8, 64
TMP = HEADS * HID            # 512
N = H * W                    # 4096
PAD = 66                     # 64 + 2 halo
NPAD = PAD * PAD             # 4356
NT = 32                      # token tiles of 128
NCH = 8                      # 512-wide chunks of the token dim
SCALE = float(HID) ** -0.5

_CACHE = {}


def _build(debug=False):
    from contextlib import ExitStack

    import concourse.bass as bass
    import concourse.mybir as mybir
    import concourse.tile as tile
    from concourse import bacc

    dt = mybir.dt
    f32, bf16 = dt.float32, dt.bfloat16
    Al = mybir.AluOpType
    Act = mybir.ActivationFunctionType

    nc = bacc.Bacc(
        "TRN2", target_bir_lowering=False, debug=False, enable_asserts=False
    )

    din = {}
    for name, shape, d in [
        ("fpa", [128, 2, NPAD], bf16),       # pad(1,1): x data at cols 1..64
        ("fpb", [128, 2, NPAD], bf16),       # pad(2,0): x data at cols 2..65
        ("wq", [128, 2, TMP], bf16),         # Wq^T   [c, o]
        ("wkv", [128, 2, 2 * TMP], bf16),    # Wkv^T  [c, o]
        ("wout", [128, 4, C], bf16),         # Wout^T [o, c]
        ("wdw", [128, 2, 9], f32),           # depthwise taps per channel
        ("bout2", [128, 2], f32),            # bias, c-tiled columns
        ("bdiag", [128, 128], bf16),         # [[J,0],[0,J]] 64x64 ones blocks
        ("onescol", [128, 1], bf16),
        ("ones11", [1, 1], f32),
    ]:
        din[name] = nc.dram_tensor(name, shape, d, kind="ExternalInput").ap()
    out_d = nc.dram_tensor("out", [2, 128, N], f32, kind="ExternalOutput").ap()
    dbg = {}
    if debug:
        for name, shape, d in [
            ("d_dw", [128, 2, N], bf16),
            ("d_expq", [128, 4, N], bf16),
            ("d_expk", [128, NT, 512], bf16),
            ("d_vsb", [128, NT, 512], bf16),
            ("d_skrow", [1, 512], f32),
            ("d_rsk", [128, 4], f32),
            ("d_ctxn", [128, 4, 128], bf16),
            ("d_eqn", [128, 4, N], bf16),
        ]:
            dbg[name] = nc.dram_tensor(
                name, shape, d, kind="ExternalOutput").ap()

    with tile.TileContext(nc) as tc, ExitStack() as ctx:
        wp = ctx.enter_context(tc.tile_pool(name="wp", bufs=1))
        sb = ctx.enter_context(tc.tile_pool(name="sb", bufs=1))

        # ---- constants / weights -------------------------------------------
        wq = wp.tile([128, 2, TMP], bf16)
        wkv = wp.tile([128, 2, 2 * TMP], bf16)
        wout = wp.tile([128, 4, C], bf16)
        wdw = wp.tile([128, 2, 9], f32)
        bout2 = wp.tile([128, 2], f32)
        bdiag = wp.tile([128, 128], bf16)
        onescol = wp.tile([128, 1], bf16)
        ones11 = wp.tile([1, 1], f32)
        for t, name in [
            (wq, "wq"), (wkv, "wkv"), (wout, "wout"), (wdw, "wdw"),
            (bout2, "bout2"), (bdiag, "bdiag"), (onescol, "onescol"),
            (ones11, "ones11"),
        ]:
            nc.sync.dma_start(out=t, in_=din[name])

        fpa = sb.tile([128, 2, NPAD], bf16)
        fpb = sb.tile([128, 2, NPAD], bf16)
        nc.sync.dma_start(out=fpa, in_=din["fpa"])
        nc.sync.dma_start(out=fpb, in_=din["fpb"])

        # ---- big sbuf tensors ----------------------------------------------
        dw = sb.tile([128, 2, N], bf16)       # depthwise output, channel-major
        expq = sb.tile([128, 4, N], bf16)     # exp(q); later reused as g
        expk = sb.tile([128, NT, 512], bf16)  # token-major
        vsb = sb.tile([128, NT, 512], bf16)   # token-major
        ctxn = sb.tile([128, 4, 128], bf16)   # block-diag scaled ctx per pair
        skrow = sb.tile([1, 512], f32)
        rsk = sb.tile([128, 4], f32)

        def fview(t, ct, dy, dx):
            # padded image view [128, 64, 64] for tap (dy, dx)
            x0 = 1 + dx if dx != 0 else 2
            src = t if dx != 0 else fpb
            im = src[:, ct].rearrange("p (y x) -> p y x", y=PAD)
            return im[:, 1 + dy:65 + dy, x0:x0 + 64]

        ctxA = ctx.enter_context(ExitStack())
        phA = ctxA.enter_context(
            tc.tile_pool(name="phA", bufs=3, space="PSUM"))
        phC = ctxA.enter_context(
            tc.tile_pool(name="phC", bufs=4, space="PSUM"))
        phS = ctxA.enter_context(
            tc.tile_pool(name="phS", bufs=1, space="PSUM"))

        # ---- depthwise 3x3 (DVE) -------------------------------------------
        taps = [(dy, dx) for dy in (-1, 0, 1) for dx in (-1, 0, 1)]
        for ct in range(2):
            dwv = dw[:, ct].rearrange("p (y x) -> p y x", y=64)
            dy, dx = taps[0]
            nc.vector.tensor_scalar_mul(
                dwv, fview(fpa, ct, dy, dx), wdw[:, ct, 0:1])
            for i, (dy, dx) in enumerate(taps[1:], start=1):
                nc.vector.scalar_tensor_tensor(
                    out=dwv, in0=fview(fpa, ct, dy, dx),
                    scalar=wdw[:, ct, i:i + 1], in1=dwv,
                    op0=Al.mult, op1=Al.add)

        # ---- q projection + exp (channel-major) ----------------------------
        for ot in range(4):
            osl = slice(ot * 128, (ot + 1) * 128)
            for ch in range(NCH):
                ps = phA.tile([128, 512], f32, tag="ps")
                for ct in range(2):
                    rhs = fpa[:, ct].rearrange("p (y x) -> p y x", y=PAD)[
                        :, 1 + 8 * ch:9 + 8 * ch, 1:65]
                    nc.tensor.matmul(
                        ps, wq[:, ct, osl], rhs,
                        start=(ct == 0), stop=(ct == 1))
                nc.scalar.activation(
                    expq[:, ot, ch * 512:(ch + 1) * 512], ps, Act.Exp)
        if debug:
            nc.sync.dma_start(out=dbg["d_expq"], in_=expq)
            nc.sync.dma_start(out=dbg["d_dw"], in_=dw)

        # ---- kv projection (token-major) + exp_k / v / Sk / ctx ------------
        skps = phA.tile([1, 512], f32, tag="ps")
        ctxps = [phC.tile([128, 128], f32, tag="ctx", name=f"ctxps{i}")
                 for i in range(4)]
        for tt in range(NT):
            tsl = slice(tt * 128, (tt + 1) * 128)
            for half in range(2):
                ps = phA.tile([128, 512], f32, tag="ps")
                hs = slice(half * 512, (half + 1) * 512)
                for ct in range(2):
                    nc.tensor.matmul(
                        ps, dw[:, ct, tsl], wkv[:, ct, hs],
                        start=(ct == 0), stop=(ct == 1))
                if half == 0:  # k
                    nc.scalar.activation(expk[:, tt], ps, Act.Exp)
                    nc.tensor.matmul(
                        skps, onescol, expk[:, tt],
                        start=(tt == 0), stop=(tt == NT - 1),
                        skip_group_check=True)
                else:          # v
                    if tt % 2 == 0:
                        nc.vector.tensor_copy(vsb[:, tt], ps)
                    else:
                        nc.scalar.copy(vsb[:, tt], ps)
            for pr in range(4):
                psl = slice(pr * 128, (pr + 1) * 128)
                nc.tensor.matmul(
                    ctxps[pr], expk[:, tt, psl], vsb[:, tt, psl],
                    start=(tt == 0), stop=(tt == NT - 1),
                    skip_group_check=True)

        # ---- Sk -> per-partition reciprocal --------------------------------
        nc.vector.tensor_copy(skrow, skps)
        for j in range(4):
            sktp = phS.tile([128, 1], f32, tag="skt")
            nc.tensor.matmul(
                sktp, skrow[0:1, j * 128:(j + 1) * 128], ones11,
                start=True, stop=True)
            nc.vector.reciprocal(rsk[:, j:j + 1], sktp)
        if debug:
            nc.sync.dma_start(out=dbg["d_expk"], in_=expk)
            nc.sync.dma_start(out=dbg["d_vsb"], in_=vsb)
            nc.sync.dma_start(out=dbg["d_skrow"], in_=skrow)
            nc.sync.dma_start(out=dbg["d_rsk"], in_=rsk)

        # ---- ctxn: scale rows, write into block-diag tile ------------------
        nc.vector.memset(ctxn, 0.0)
        for pr in range(4):
            for hh in range(2):
                rs = slice(hh * 64, (hh + 1) * 64)
                cs = slice(pr * 128 + hh * 64, pr * 128 + hh * 64 + 64)
                nc.vector.tensor_scalar(
                    out=ctxn[rs, pr, hh * 64:hh * 64 + 64],
                    in0=ctxps[pr][rs, hh * 64:hh * 64 + 64],
                    scalar1=rsk[rs, pr:pr + 1], scalar2=SCALE,
                    op0=Al.mult, op1=Al.mult)
        if debug:
            nc.sync.dma_start(out=dbg["d_ctxn"], in_=ctxn)

        # ---- phase B: Sq, eqn, att, gelu, out projection -------------------
        ctxA.close()  # release phase-A PSUM pools
        eqp = ctx.enter_context(tc.tile_pool(name="eqp", bufs=2))
        rbp = ctx.enter_context(tc.tile_pool(name="rbp", bufs=2))
        phQ = ctx.enter_context(
            tc.tile_pool(name="phQ", bufs=1, space="PSUM"))
        phB = ctx.enter_context(
            tc.tile_pool(name="phB", bufs=3, space="PSUM"))

        for ot in range(4):
            for r in range(2):
                rsl = slice(r * 2048, (r + 1) * 2048)
                sq = phQ.tile([128, 2048], f32, tag="sq")
                for c in range(4):
                    nc.tensor.matmul(
                        sq[:, c * 512:(c + 1) * 512], bdiag,
                        expq[:, ot, r * 2048 + c * 512:r * 2048 + (c + 1) * 512],
                        start=True, stop=True)
                rb = rbp.tile([128, 2048], bf16, tag="rb")
                with nc.allow_low_precision("softmax denominators fit bf16"):
                    nc.vector.reciprocal(rb, sq)
                eqn = eqp.tile([128, 2048], bf16, tag="eqn")
                nc.vector.tensor_mul(eqn, expq[:, ot, rsl], rb)
                if debug:
                    nc.sync.dma_start(out=dbg["d_eqn"][:, ot, rsl], in_=eqn)
                for c in range(4):
                    att = phB.tile([128, 512], f32, tag="att")
                    nc.tensor.matmul(
                        att, ctxn[:, ot], eqn[:, c * 512:(c + 1) * 512],
                        start=True, stop=True)
                    gsl = slice(r * 2048 + c * 512, r * 2048 + (c + 1) * 512)
                    nc.scalar.activation(expq[:, ot, gsl], att, Act.Gelu)

        osb = ctx.enter_context(tc.tile_pool(name="osb", bufs=3))
        for ct in range(2):
            for ch in range(NCH):
                ps = phB.tile([128, 512], f32, tag="att")
                csl = slice(ch * 512, (ch + 1) * 512)
                for ot in range(4):
                    nc.tensor.matmul(
                        ps, wout[:, ot, ct * 128:(ct + 1) * 128],
                        expq[:, ot, csl], start=(ot == 0), stop=(ot == 3))
                ot_sb = osb.tile([128, 512], f32, tag="osb")
                nc.scalar.activation(
                    ot_sb, ps, Act.Identity, bias=bout2[:, ct:ct + 1])
                nc.sync.dma_start(out=out_d[ct, :, csl], in_=ot_sb)

    nc.compile()
    return nc


def _prep_inputs(fmap, Wq, Wdw, Wkv, Wout, bout):
    bf16 = ml_dtypes.bfloat16
    f32 = np.float32

    def ctile(a):  # [256, X] -> [128, 2, X]
        return np.ascontiguousarray(
            a.reshape(2, 128, *a.shape[1:]).transpose(1, 0, *range(2, a.ndim + 1)))

    shared = {
        "wq": ctile(Wq.T.astype(bf16)),
        "wkv": ctile(Wkv.T.astype(bf16)),
        "wout": np.ascontiguousarray(
            Wout.T.astype(bf16).reshape(4, 128, C).transpose(1, 0, 2)),
        "wdw": ctile(Wdw.reshape(C, 9).astype(f32)),
        "bout2": np.ascontiguousarray(bout.astype(f32).reshape(2, 128).T),
        "bdiag": np.kron(np.eye(2, dtype=f32), np.ones((64, 64), f32)).astype(bf16),
        "onescol": np.ones((128, 1), bf16),
        "ones11": np.ones((1, 1), f32),
    }
    in_maps = []
    for b in range(B):
        fpa = np.pad(fmap[b], [(0, 0), (1, 1), (1, 1)]).astype(bf16)
        fpb = np.pad(fmap[b], [(0, 0), (1, 1), (2, 0)]).astype(bf16)
        m = dict(shared)
        m["fpa"] = ctile(fpa.reshape(C, NPAD))
        m["fpb"] = ctile(fpb.reshape(C, NPAD))
        in_maps.append(m)
    return in_maps


def kernel(fmap, Wq, Wdw, Wkv, Wout, bout, _trace=False, _tmpdir=None):
    from concourse.bass_utils import run_bass_kernel_spmd

    fmap, Wq, Wdw, Wkv, Wout, bout = (
        np.asarray(a, np.float32) for a in (fmap, Wq, Wdw, Wkv, Wout, bout))

    if "nc" not in _CACHE:
        _CACHE["nc"] = _build()
    nc = _CACHE["nc"]

    in_maps = _prep_inputs(fmap, Wq, Wdw, Wkv, Wout, bout)
    res = run_bass_kernel_spmd(
        nc, in_maps, core_ids=list(range(B)), trace=_trace, tmpdir=_tmpdir)
    _CACHE["last_result"] = res
    out = np.stack([r["out"] for r in res.results])        # [B, 2, 128, N]
    return out.reshape(B, C, H, W).astype(np.float32)



# revision 8
# speedup vs baseline: 1.8286x; 1.8286x over previous
"""ConvAttention (linear attention with conv projections) on 8 trn2 cores.

Sharding: data-parallel over batch B=8, one image per NeuronCore.

v2 pipeline (channel-major activations [chan, tok], tok = y*64+x):
  qproj   PE -> exp (ACT, [128,1024] tiles) -> expq sbuf
  dw      depthwise3x3: ct0 on GPSIMD (9 STT), ct1 on DVE (TS 4x + TT-add 2x),
          chunked by 16 y-rows to overlap with the kv chain
  kv      PE token-major psum [128,1024]; exp_k (ACT) -> ring; v copy -> ring
  ctx     PE [128,129] blocks: v augmented with ones column so col 128
          accumulates Sk per partition (no separate row-sum / transpose)
  ctxn    DVE scale by 1/Sk * scale into block-diag tile
  Sq      PE bdiag matmul -> psum; 1/Sq on ACT Reciprocal LUT (batched after
          all exps: one table swap) -> rb sbuf
  eqn     DVE expq * rb
  att     PE ctxn^T @ eqn
  gelu    ACT psum -> bf16 sbuf (reuses expq storage)
  out     PE Wout @ g + bias via K=1 ones matmul; DMA psum -> DRAM f32
"""

import numpy as np
import ml_dtypes

B, C, H, W = 8, 256, 64, 64
HEADS, HID = 8, 64
TMP = HEADS * HID            # 512
N = H * W                    # 4096
PAD = 66                     # 64 + 2 halo
NPAD = PAD * PAD             # 4356
NT = 32                      # token tiles of 128
NCHUNK = 4                   # dw/kv chunks of 16 y-rows (1024 tokens)
RING = 4                     # expk / vsb ring depth (token tiles)
SCALE = float(HID) ** -0.5

_CACHE = {}


def _build(debug=False):
    from contextlib import ExitStack

    import concourse.bass as bass
    import concourse.mybir as mybir
    import concourse.tile as tile
    from concourse import bacc

    dt = mybir.dt
    f32, bf16 = dt.float32, dt.bfloat16
    Al = mybir.AluOpType
    Act = mybir.ActivationFunctionType

    nc = bacc.Bacc(
        "TRN2", target_bir_lowering=False, debug=False, enable_asserts=False
    )

    din = {}
    for name, shape, d in [
        ("fpa", [128, 2, NPAD], bf16),       # pad(1,1): x data at cols 1..64
        ("fpb", [128, 2, NPAD], bf16),       # pad(2,0): x data at cols 2..65
        ("wq", [128, 2, TMP], bf16),         # Wq^T   [c, o]
        ("wkv", [128, 2, 2 * TMP], bf16),    # Wkv^T  [c, o]
        ("wout", [128, 4, C], bf16),         # Wout^T [o, c]
        ("wdw", [128, 9], f32),              # depthwise taps, ct1 channels
        ("wdiag", [128, 9, 128], bf16),      # diag(tap) matrices, ct0 chans
        ("bout2", [128, 2], f32),            # bias, c-tiled columns
        ("bdiag", [128, 128], bf16),         # [[J,0],[0,J]] 64x64 ones blocks
    ]:
        din[name] = nc.dram_tensor(name, shape, d, kind="ExternalInput").ap()
    out_d = nc.dram_tensor("out", [2, 128, N], f32, kind="ExternalOutput").ap()
    dbg = {}
    if debug:
        for name, shape, d in [
            ("d_dw", [128, 2, N], bf16),
            ("d_expq", [128, 4, N], bf16),
            ("d_rsk", [128, 4], f32),
            ("d_ctxn", [128, 4, 128], bf16),
            ("d_rb", [128, 4, N], bf16),
        ]:
            dbg[name] = nc.dram_tensor(
                name, shape, d, kind="ExternalOutput").ap()

    with tile.TileContext(nc) as tc, ExitStack() as ctx:
        wp = ctx.enter_context(tc.tile_pool(name="wp", bufs=1))
        sb = ctx.enter_context(tc.tile_pool(name="sb", bufs=1))

        # ---- constants / weights -------------------------------------------
        wq = wp.tile([128, 2, TMP], bf16)
        wkv = wp.tile([128, 2, 2 * TMP], bf16)
        wout = wp.tile([128, 4, C], bf16)
        wdw = wp.tile([128, 9], f32)
        wdiag = wp.tile([128, 9, 128], bf16)
        bout2 = wp.tile([128, 2], f32)
        bdiag = wp.tile([128, 128], bf16)
        # input images in 4 row-bands so early chunks start sooner; the
        # first band and the weights feeding the first matmuls go first
        fpa = sb.tile([128, 2, NPAD], bf16)
        fpb = sb.tile([128, 2, NPAD], bf16)
        bands = [(0, 18), (18, 34), (34, 50), (50, 66)]

        def band_dma(r0, r1):
            sl = slice(r0 * PAD, r1 * PAD)
            nc.sync.dma_start(out=fpa[:, :, sl], in_=din["fpa"][:, :, sl])
            nc.sync.dma_start(out=fpb[:, :, sl], in_=din["fpb"][:, :, sl])

        band_dma(*bands[0])
        for t, name in [(wq, "wq"), (wdw, "wdw"), (wdiag, "wdiag")]:
            nc.sync.dma_start(out=t, in_=din[name])
        band_dma(*bands[1])
        for t, name in [(wkv, "wkv"), (wout, "wout"),
                        (bout2, "bout2"), (bdiag, "bdiag")]:
            nc.sync.dma_start(out=t, in_=din[name])
        for b in bands[2:]:
            band_dma(*b)

        # ---- big sbuf tensors ----------------------------------------------
        dw = sb.tile([128, 2, N], bf16)       # depthwise output, channel-major
        tmpv = sb.tile([128, 1024], bf16)     # DVE tap staging
        expq = sb.tile([128, 4, N], bf16)     # exp(q); later reused as g
        expk = sb.tile([128, RING, 512], bf16)   # token-major ring
        vsb = sb.tile([128, RING, 4, 132], bf16)  # v ring + ones col at 128
        ctxn = sb.tile([128, 4, 128], bf16)   # block-diag scaled ctx per pair
        rsk = sb.tile([128, 4], f32)
        rb = sb.tile([128, 4, N], bf16)       # 1/Sq broadcast per head pair

        nc.gpsimd.memset(vsb[:, :, :, 128:129], 1.0)
        nc.gpsimd.memset(ctxn, 0.0)

        def fview(ct, dy, dx, y0, ny):
            # padded image view [128, ny, 64] for tap (dy, dx), rows y0..y0+ny
            x0 = 1 + dx if dx != 0 else 2
            src = fpa if dx != 0 else fpb
            im = src[:, ct].rearrange("p (y x) -> p y x", y=PAD)
            return im[:, 1 + dy + y0:1 + dy + y0 + ny, x0:x0 + 64]

        def qview(ct, y0, ny):
            im = fpa[:, ct].rearrange("p (y x) -> p y x", y=PAD)
            return im[:, 1 + y0:1 + y0 + ny, 1:65]

        ctxA = ctx.enter_context(ExitStack())
        pa = ctxA.enter_context(
            tc.tile_pool(name="pa", bufs=3, space="PSUM"))
        phC = ctxA.enter_context(
            tc.tile_pool(name="phC", bufs=2, space="PSUM"))
        ctxt = [phC.tile([128, 2, 129], f32, tag="ctx", name=f"ctxt{i}")
                for i in range(2)]

        taps = [(dy, dx) for dy in (-1, 0, 1) for dx in (-1, 0, 1)]

        # ---- q projection + exp (channel-major), chunk-inner ---------------
        for ot in range(4):
            osl = slice(ot * 128, (ot + 1) * 128)
            for ch in range(NCHUNK):
                ps = pa.tile([128, 1024], f32, tag="pa")
                for hf in range(2):
                    y0 = ch * 16 + hf * 8
                    for ct in range(2):
                        nc.tensor.matmul(
                            ps[:, hf * 512:(hf + 1) * 512],
                            wq[:, ct, osl], qview(ct, y0, 8),
                            start=(ct == 0), stop=(ct == 1))
                nc.scalar.activation(
                    expq[:, ot, ch * 1024:(ch + 1) * 1024], ps, Act.Exp)

        # ---- depthwise + kv + ctx, pipelined chunk emission ----------------
        def emit_dw(ch):
            y0 = ch * 16
            csl = slice(ch * 1024, (ch + 1) * 1024)
            # ct0 on PE: 9 diag-matmul taps accumulated in psum, ACT copy out
            dwp = pa.tile([128, 1024], f32, tag="pa")
            for i, (dy, dx) in enumerate(taps):
                for hf in range(2):
                    nc.tensor.matmul(
                        dwp[:, hf * 512:(hf + 1) * 512], wdiag[:, i],
                        fview(0, dy, dx, y0 + hf * 8, 8),
                        start=(i == 0), stop=(i == 8))
            nc.scalar.copy(dw[:, 0, csl], dwp)
            # ct1 on DVE: tensor_scalar 4x + tensor_tensor add 2x
            dwv = dw[:, 1, csl].rearrange("p (y x) -> p y x", y=16)
            tmp3 = tmpv.rearrange("p (y x) -> p y x", y=16)
            for i, (dy, dx) in enumerate(taps):
                fv = fview(1, dy, dx, y0, 16)
                if i == 0:
                    nc.vector.tensor_scalar_mul(dwv, fv, wdw[:, 0:1])
                else:
                    nc.vector.tensor_scalar_mul(tmp3, fv, wdw[:, i:i + 1])
                    nc.vector.tensor_add(dwv, dwv, tmp3)

        def emit_kv(ch):
            for tt in range(ch * 8, ch * 8 + 8):
                tsl = slice(tt * 128, (tt + 1) * 128)
                r = tt % RING
                ps = pa.tile([128, 1024], f32, tag="pa")
                for ct in range(2):
                    nc.tensor.matmul(
                        ps[:, 0:512], dw[:, ct, tsl], wkv[:, ct, 0:512],
                        start=(ct == 0), stop=(ct == 1))
                    nc.tensor.matmul(
                        ps[:, 512:1024], dw[:, ct, tsl], wkv[:, ct, 512:1024],
                        start=(ct == 0), stop=(ct == 1))
                nc.scalar.activation(expk[:, r], ps[:, 0:512], Act.Exp)
                vdst = vsb[:, r, :, 0:128]
                vsrc = ps[:, 512:1024].rearrange("p (a b) -> p a b", a=4)
                if tt % 2 == 0:
                    nc.vector.tensor_copy(vdst, vsrc)
                else:
                    nc.scalar.copy(vdst, vsrc)
                for pr in range(4):
                    psl = slice(pr * 128, (pr + 1) * 128)
                    # start=True zeroes the whole 2KB psum bank; only the
                    # first region per bank may use it (pr%2==1 accumulates
                    # onto the bank just zeroed by its pr%2==0 sibling).
                    nc.tensor.matmul(
                        ctxt[pr // 2][:, pr % 2], expk[:, r, psl],
                        vsb[:, r, pr, 0:129],
                        start=(tt == 0 and pr % 2 == 0),
                        stop=(tt == NT - 1),
                        skip_group_check=True)

        emit_dw(0)
        for ch in range(1, NCHUNK):
            emit_dw(ch)
            emit_kv(ch - 1)
        emit_kv(NCHUNK - 1)

        # ---- Sk reciprocal + ctxn block-diag build -------------------------
        for pr in range(4):
            nc.vector.reciprocal(
                rsk[:, pr:pr + 1], ctxt[pr // 2][:, pr % 2, 128:129])
        for pr in range(4):
            for hh in range(2):
                rs = slice(hh * 64, (hh + 1) * 64)
                nc.vector.tensor_scalar(
                    out=ctxn[rs, pr, hh * 64:hh * 64 + 64],
                    in0=ctxt[pr // 2][rs, pr % 2, hh * 64:hh * 64 + 64],
                    scalar1=rsk[rs, pr:pr + 1], scalar2=SCALE,
                    op0=Al.mult, op1=Al.mult)
        if debug:
            nc.sync.dma_start(out=dbg["d_dw"], in_=dw)
            nc.sync.dma_start(out=dbg["d_expq"], in_=expq)
            nc.sync.dma_start(out=dbg["d_rsk"], in_=rsk)
            nc.sync.dma_start(out=dbg["d_ctxn"], in_=ctxn)

        # ---- phase B1: Sq via bdiag matmul; 1/Sq approx on DVE; cast on GP --
        ctxA.close()  # release pa + phC
        phS = ctx.enter_context(
            tc.tile_pool(name="phS", bufs=2, space="PSUM"))
        rbp32 = ctx.enter_context(tc.tile_pool(name="rbp32", bufs=2))
        for ot in range(4):
            rb32 = rbp32.tile([128, N], f32, tag="rb32")
            for j in range(4):
                base = j * 1024
                sqt = phS.tile([128, 1024], f32, tag="sq")
                for hf in range(2):
                    nc.tensor.matmul(
                        sqt[:, hf * 512:(hf + 1) * 512], bdiag,
                        expq[:, ot, base + hf * 512:base + (hf + 1) * 512],
                        start=True, stop=True)
                nc.vector.reciprocal_approx_fast(
                    out=rb32[:, base:base + 1024], in_=sqt)
                nc.vector.tensor_copy(
                    rb[:, ot, base:base + 1024], rb32[:, base:base + 1024])
        if debug:
            nc.sync.dma_start(out=dbg["d_rb"], in_=rb)

        # ---- phase B2: eqn, att, gelu, out projection ----------------------
        eqp = ctx.enter_context(tc.tile_pool(name="eqp", bufs=3))
        attp = ctx.enter_context(
            tc.tile_pool(name="attp", bufs=2, space="PSUM"))
        outp = ctx.enter_context(
            tc.tile_pool(name="outp", bufs=2, space="PSUM"))
        osb = ctx.enter_context(tc.tile_pool(name="osb", bufs=3))
        for c in range(8):
            csl = slice(c * 512, (c + 1) * 512)
            for ot in range(4):
                eqn = eqp.tile([128, 512], bf16, tag="eqn")
                nc.vector.tensor_mul(eqn, expq[:, ot, csl], rb[:, ot, csl])
                att = attp.tile([128, 512], f32, tag="att")
                nc.tensor.matmul(att, ctxn[:, ot], eqn, start=True, stop=True)
                nc.scalar.activation(expq[:, ot, csl], att, Act.Gelu)
            for ct in range(2):
                op = outp.tile([128, 512], f32, tag="op")
                for ot in range(4):
                    nc.tensor.matmul(
                        op, wout[:, ot, ct * 128:(ct + 1) * 128],
                        expq[:, ot, csl], start=(ot == 0), stop=(ot == 3))
                ot_sb = osb.tile([128, 512], f32, tag="osb")
                nc.scalar.activation(
                    ot_sb, op, Act.Identity, bias=bout2[:, ct:ct + 1])
                nc.sync.dma_start(out=out_d[ct, :, csl], in_=ot_sb)

    nc.compile()
    return nc


def _prep_inputs(fmap, Wq, Wdw, Wkv, Wout, bout):
    bf16 = ml_dtypes.bfloat16
    f32 = np.float32

    def ctile(a):  # [256, X] -> [128, 2, X]
        return np.ascontiguousarray(
            a.reshape(2, 128, *a.shape[1:]).transpose(1, 0, *range(2, a.ndim + 1)))

    shared = {
        "wq": ctile(Wq.T.astype(bf16)),
        "wkv": ctile(Wkv.T.astype(bf16)),
        "wout": np.ascontiguousarray(
            Wout.T.astype(bf16).reshape(4, 128, C).transpose(1, 0, 2)),
        "wdw": np.ascontiguousarray(Wdw.reshape(C, 9)[128:].astype(f32)),
        "wdiag": np.ascontiguousarray(
            np.stack([np.diag(Wdw.reshape(C, 9)[:128, i]) for i in range(9)],
                     axis=1).astype(bf16)),
        "bout2": np.ascontiguousarray(bout.astype(f32).reshape(2, 128).T),
        "bdiag": np.kron(np.eye(2, dtype=f32), np.ones((64, 64), f32)).astype(bf16),
    }
    in_maps = []
    for b in range(B):
        fpa = np.pad(fmap[b], [(0, 0), (1, 1), (1, 1)]).astype(bf16)
        fpb = np.pad(fmap[b], [(0, 0), (1, 1), (2, 0)]).astype(bf16)
        m = dict(shared)
        m["fpa"] = ctile(fpa.reshape(C, NPAD))
        m["fpb"] = ctile(fpb.reshape(C, NPAD))
        in_maps.append(m)
    return in_maps


def kernel(fmap, Wq, Wdw, Wkv, Wout, bout, _trace=False, _tmpdir=None,
           _debug=False):
    from concourse.bass_utils import run_bass_kernel_spmd

    fmap, Wq, Wdw, Wkv, Wout, bout = (
        np.asarray(a, np.float32) for a in (fmap, Wq, Wdw, Wkv, Wout, bout))

    key = "nc_dbg" if _debug else "nc"
    if key not in _CACHE:
        _CACHE[key] = _build(debug=_debug)
    nc = _CACHE[key]

    in_maps = _prep_inputs(fmap, Wq, Wdw, Wkv, Wout, bout)
    res = run_bass_kernel_spmd(
        nc, in_maps, core_ids=list(range(B)), trace=_trace, tmpdir=_tmpdir)
    _CACHE["last_result"] = res
    out = np.stack([r["out"] for r in res.results])        # [B, 2, 128, N]
    return out.reshape(B, C, H, W).astype(np.float32)


# revision 9
# speedup vs baseline: 2.1112x; 1.1545x over previous
"""ConvAttention (linear attention with conv projections) on 8 trn2 cores.

Sharding: data-parallel over batch B=8, one image per NeuronCore.

v2 pipeline (channel-major activations [chan, tok], tok = y*64+x):
  qproj   PE -> exp (ACT, [128,1024] tiles) -> expq sbuf
  dw      depthwise3x3: ct0 on GPSIMD (9 STT), ct1 on DVE (TS 4x + TT-add 2x),
          chunked by 16 y-rows to overlap with the kv chain
  kv      PE token-major psum [128,1024]; exp_k (ACT) -> ring; v copy -> ring
  ctx     PE [128,129] blocks: v augmented with ones column so col 128
          accumulates Sk per partition (no separate row-sum / transpose)
  ctxn    DVE scale by 1/Sk * scale into block-diag tile
  Sq      PE bdiag matmul -> psum; 1/Sq on ACT Reciprocal LUT (batched after
          all exps: one table swap) -> rb sbuf
  eqn     DVE expq * rb
  att     PE ctxn^T @ eqn
  gelu    ACT psum -> bf16 sbuf (reuses expq storage)
  out     PE Wout @ g + bias via K=1 ones matmul; DMA psum -> DRAM f32
"""

import numpy as np
import ml_dtypes

B, C, H, W = 8, 256, 64, 64
HEADS, HID = 8, 64
TMP = HEADS * HID            # 512
N = H * W                    # 4096
PAD = 66                     # 64 + 2 halo
NPAD = PAD * PAD             # 4356
NT = 32                      # token tiles of 128
NCHUNK = 4                   # dw/kv chunks of 16 y-rows (1024 tokens)
RING = 4                     # expk / vsb ring depth (token tiles)
SCALE = float(HID) ** -0.5

_CACHE = {}


def _build(debug=False):
    from contextlib import ExitStack

    import concourse.bass as bass
    import concourse.mybir as mybir
    import concourse.tile as tile
    from concourse import bacc

    dt = mybir.dt
    f32, bf16 = dt.float32, dt.bfloat16
    Al = mybir.AluOpType
    Act = mybir.ActivationFunctionType

    nc = bacc.Bacc(
        "TRN2", target_bir_lowering=False, debug=False, enable_asserts=False
    )

    din = {}
    for name, shape, d in [
        ("fpa", [128, 2, NPAD], bf16),       # pad(1,1): x data at cols 1..64
        ("fpb", [128, 2, NPAD], bf16),       # pad(2,0): x data at cols 2..65
        ("wq", [128, 2, TMP], bf16),         # Wq^T   [c, o]
        ("wkv", [128, 2, 2 * TMP], bf16),    # Wkv^T  [c, o]
        ("wout", [128, 4, C], bf16),         # Wout^T [o, c]
        ("wdw", [128, 9], f32),              # depthwise taps, ct1 channels
        ("wdiag", [128, 9, 128], bf16),      # diag(tap) matrices, ct0 chans
        ("boutT", [1, C], bf16),             # bias as single-partition row
        ("bdiag", [128, 128], bf16),         # [[J,0],[0,J]] 64x64 ones blocks
    ]:
        din[name] = nc.dram_tensor(name, shape, d, kind="ExternalInput").ap()
    out_d = nc.dram_tensor("out", [2, 128, N], f32, kind="ExternalOutput").ap()
    dbg = {}
    if debug:
        for name, shape, d in [
            ("d_dw", [128, 2, N], bf16),
            ("d_expq", [128, 4, N], bf16),
            ("d_rsk", [128, 4], f32),
            ("d_ctxn", [128, 4, 128], bf16),
            ("d_rb", [128, 4, N], bf16),
        ]:
            dbg[name] = nc.dram_tensor(
                name, shape, d, kind="ExternalOutput").ap()

    with tile.TileContext(nc) as tc, ExitStack() as ctx:
        wp = ctx.enter_context(tc.tile_pool(name="wp", bufs=1))
        sb = ctx.enter_context(tc.tile_pool(name="sb", bufs=1))

        # ---- constants / weights -------------------------------------------
        wq = wp.tile([128, 2, TMP], bf16)
        wkv = wp.tile([128, 2, 2 * TMP], bf16)
        wout = wp.tile([128, 4, C], bf16)
        wdw = wp.tile([128, 9], f32)
        wdiag = wp.tile([128, 9, 128], bf16)
        boutT = wp.tile([1, C], bf16)
        bdiag = wp.tile([128, 128], bf16)
        # input images in 4 row-bands so early chunks start sooner; the
        # first band and the weights feeding the first matmuls go first
        fpa = sb.tile([128, 2, NPAD], bf16)
        fpb = sb.tile([128, 2, NPAD], bf16)
        bands = [(0, 18), (18, 34), (34, 50), (50, 66)]

        def band_dma(r0, r1):
            sl = slice(r0 * PAD, r1 * PAD)
            nc.sync.dma_start(out=fpa[:, :, sl], in_=din["fpa"][:, :, sl])
            nc.sync.dma_start(out=fpb[:, :, sl], in_=din["fpb"][:, :, sl])

        band_dma(*bands[0])
        for t, name in [(wq, "wq"), (wdw, "wdw"), (wdiag, "wdiag")]:
            nc.sync.dma_start(out=t, in_=din[name])
        band_dma(*bands[1])
        for t, name in [(wkv, "wkv"), (wout, "wout"),
                        (boutT, "boutT"), (bdiag, "bdiag")]:
            nc.sync.dma_start(out=t, in_=din[name])
        for b in bands[2:]:
            band_dma(*b)

        # ---- big sbuf tensors ----------------------------------------------
        dw = sb.tile([128, 2, N], bf16)       # depthwise output, channel-major
        tmpv = sb.tile([128, 1024], bf16)     # DVE tap staging
        expq = sb.tile([128, 4, N], bf16)     # exp(q); later reused as g
        expk = sb.tile([128, RING, 512], bf16)   # token-major ring
        vsb = sb.tile([128, RING, 4, 132], bf16)  # v ring + ones col at 128
        ctxn = sb.tile([128, 4, 128], bf16)   # block-diag scaled ctx per pair
        onesrow = sb.tile([1, 512], bf16)
        rsk = sb.tile([128, 4], f32)
        rb = sb.tile([128, 4, N], bf16)       # 1/Sq broadcast per head pair

        nc.gpsimd.memset(vsb[:, :, :, 128:129], 1.0)
        nc.gpsimd.memset(onesrow, 1.0)
        nc.gpsimd.memset(ctxn, 0.0)

        def fview(ct, dy, dx, y0, ny):
            # padded image view [128, ny, 64] for tap (dy, dx), rows y0..y0+ny
            x0 = 1 + dx if dx != 0 else 2
            src = fpa if dx != 0 else fpb
            im = src[:, ct].rearrange("p (y x) -> p y x", y=PAD)
            return im[:, 1 + dy + y0:1 + dy + y0 + ny, x0:x0 + 64]

        def qview(ct, y0, ny):
            im = fpa[:, ct].rearrange("p (y x) -> p y x", y=PAD)
            return im[:, 1 + y0:1 + y0 + ny, 1:65]

        ctxA = ctx.enter_context(ExitStack())
        pa = ctxA.enter_context(
            tc.tile_pool(name="pa", bufs=2, space="PSUM"))
        phC = ctxA.enter_context(
            tc.tile_pool(name="phC", bufs=2, space="PSUM"))
        ctxt = [phC.tile([128, 2, 129], f32, tag="ctx", name=f"ctxt{i}")
                for i in range(2)]

        taps = [(dy, dx) for dy in (-1, 0, 1) for dx in (-1, 0, 1)]

        # ---- q projection + exp (channel-major), chunk-inner ---------------
        for ot in range(4):
            osl = slice(ot * 128, (ot + 1) * 128)
            for ch in range(NCHUNK):
                ps = pa.tile([128, 1024], f32, tag="pa")
                for hf in range(2):
                    y0 = ch * 16 + hf * 8
                    for ct in range(2):
                        nc.tensor.matmul(
                            ps[:, hf * 512:(hf + 1) * 512],
                            wq[:, ct, osl], qview(ct, y0, 8),
                            start=(ct == 0), stop=(ct == 1))
                nc.scalar.activation(
                    expq[:, ot, ch * 1024:(ch + 1) * 1024], ps, Act.Exp)

        # ---- depthwise + kv + ctx, pipelined chunk emission ----------------
        def emit_dw(ch):
            y0 = ch * 16
            csl = slice(ch * 1024, (ch + 1) * 1024)
            # ct0 on PE: 9 diag-matmul taps accumulated in psum, ACT copy out
            dwp = pa.tile([128, 1024], f32, tag="pa")
            for i, (dy, dx) in enumerate(taps):
                for hf in range(2):
                    nc.tensor.matmul(
                        dwp[:, hf * 512:(hf + 1) * 512], wdiag[:, i],
                        fview(0, dy, dx, y0 + hf * 8, 8),
                        start=(i == 0), stop=(i == 8))
            nc.scalar.copy(dw[:, 0, csl], dwp)
            # ct1 on DVE: tensor_scalar 4x + tensor_tensor add 2x
            dwv = dw[:, 1, csl].rearrange("p (y x) -> p y x", y=16)
            tmp3 = tmpv.rearrange("p (y x) -> p y x", y=16)
            for i, (dy, dx) in enumerate(taps):
                fv = fview(1, dy, dx, y0, 16)
                if i == 0:
                    nc.vector.tensor_scalar_mul(dwv, fv, wdw[:, 0:1])
                else:
                    nc.vector.tensor_scalar_mul(tmp3, fv, wdw[:, i:i + 1])
                    nc.vector.tensor_add(dwv, dwv, tmp3)

        def emit_kv(ch):
            for tt in range(ch * 8, ch * 8 + 8):
                tsl = slice(tt * 128, (tt + 1) * 128)
                r = tt % RING
                ps = pa.tile([128, 1024], f32, tag="pa")
                for ct in range(2):
                    nc.tensor.matmul(
                        ps[:, 0:512], dw[:, ct, tsl], wkv[:, ct, 0:512],
                        start=(ct == 0), stop=(ct == 1))
                    nc.tensor.matmul(
                        ps[:, 512:1024], dw[:, ct, tsl], wkv[:, ct, 512:1024],
                        start=(ct == 0), stop=(ct == 1))
                nc.scalar.activation(expk[:, r], ps[:, 0:512], Act.Exp)
                vdst = vsb[:, r, :, 0:128]
                vsrc = ps[:, 512:1024].rearrange("p (a b) -> p a b", a=4)
                nc.scalar.copy(vdst, vsrc)
                for pr in range(4):
                    psl = slice(pr * 128, (pr + 1) * 128)
                    # start=True zeroes the whole 2KB psum bank; only the
                    # first region per bank may use it (pr%2==1 accumulates
                    # onto the bank just zeroed by its pr%2==0 sibling).
                    nc.tensor.matmul(
                        ctxt[pr // 2][:, pr % 2], expk[:, r, psl],
                        vsb[:, r, pr, 0:129],
                        start=(tt == 0 and pr % 2 == 0),
                        stop=(tt == NT - 1),
                        skip_group_check=True)

        phS = ctxA.enter_context(
            tc.tile_pool(name="phS", bufs=2, space="PSUM"))
        rbp32 = ctx.enter_context(tc.tile_pool(name="rbp32", bufs=2))

        def emit_b1(ot):
            # Sq via bdiag matmul; 1/Sq approx + bf16 cast on DVE
            rb32 = rbp32.tile([128, N], f32, tag="rb32")
            for j in range(8):
                base = j * 512
                sqt = phS.tile([128, 512], f32, tag="sq")
                nc.tensor.matmul(
                    sqt, bdiag, expq[:, ot, base:base + 512],
                    start=True, stop=True)
                nc.vector.reciprocal_approx_fast(
                    out=rb32[:, base:base + 512], in_=sqt)
                nc.vector.tensor_copy(
                    rb[:, ot, base:base + 512], rb32[:, base:base + 512])

        emit_dw(0)
        emit_dw(1)
        emit_kv(0)
        emit_b1(0)
        emit_dw(2)
        emit_kv(1)
        emit_b1(1)
        emit_dw(3)
        emit_kv(2)
        emit_b1(2)
        emit_kv(3)
        emit_b1(3)

        # ---- Sk reciprocal + ctxn block-diag build -------------------------
        for pr in range(4):
            nc.vector.reciprocal(
                rsk[:, pr:pr + 1], ctxt[pr // 2][:, pr % 2, 128:129])
        for pr in range(4):
            for hh in range(2):
                rs = slice(hh * 64, (hh + 1) * 64)
                nc.vector.tensor_scalar(
                    out=ctxn[rs, pr, hh * 64:hh * 64 + 64],
                    in0=ctxt[pr // 2][rs, pr % 2, hh * 64:hh * 64 + 64],
                    scalar1=rsk[rs, pr:pr + 1], scalar2=SCALE,
                    op0=Al.mult, op1=Al.mult)
        if debug:
            nc.sync.dma_start(out=dbg["d_dw"], in_=dw)
            nc.sync.dma_start(out=dbg["d_expq"], in_=expq)
            nc.sync.dma_start(out=dbg["d_rsk"], in_=rsk)
            nc.sync.dma_start(out=dbg["d_ctxn"], in_=ctxn)

        ctxA.close()  # release pa + phC + phS
        if debug:
            nc.sync.dma_start(out=dbg["d_rb"], in_=rb)

        # ---- phase B2: eqn, att, gelu, out projection ----------------------
        eqp = ctx.enter_context(tc.tile_pool(name="eqp", bufs=3))
        attp = ctx.enter_context(
            tc.tile_pool(name="attp", bufs=2, space="PSUM"))
        outp = ctx.enter_context(
            tc.tile_pool(name="outp", bufs=2, space="PSUM"))
        osb = ctx.enter_context(tc.tile_pool(name="osb", bufs=3))
        for c in range(8):
            csl = slice(c * 512, (c + 1) * 512)
            for ot in range(4):
                eqn = eqp.tile([128, 512], bf16, tag="eqn")
                nc.vector.tensor_mul(eqn, expq[:, ot, csl], rb[:, ot, csl])
                att = attp.tile([128, 512], f32, tag="att")
                nc.tensor.matmul(att, ctxn[:, ot], eqn, start=True, stop=True)
                nc.scalar.activation(expq[:, ot, csl], att, Act.Gelu)
            for ct in range(2):
                op = outp.tile([128, 512], f32, tag="op")
                for ot in range(4):
                    nc.tensor.matmul(
                        op, wout[:, ot, ct * 128:(ct + 1) * 128],
                        expq[:, ot, csl], start=(ot == 0), stop=False)
                nc.tensor.matmul(
                    op, boutT[0:1, ct * 128:(ct + 1) * 128],
                    onesrow[0:1, 0:512], start=False, stop=True)
                ot_sb = osb.tile([128, 512], f32, tag="osb")
                nc.vector.tensor_copy(ot_sb, op)
                nc.sync.dma_start(out=out_d[ct, :, csl], in_=ot_sb)

    nc.compile()
    return nc


def _prep_inputs(fmap, Wq, Wdw, Wkv, Wout, bout):
    bf16 = ml_dtypes.bfloat16
    f32 = np.float32

    def ctile(a):  # [256, X] -> [128, 2, X]
        return np.ascontiguousarray(
            a.reshape(2, 128, *a.shape[1:]).transpose(1, 0, *range(2, a.ndim + 1)))

    shared = {
        "wq": ctile(Wq.T.astype(bf16)),
        "wkv": ctile(Wkv.T.astype(bf16)),
        "wout": np.ascontiguousarray(
            Wout.T.astype(bf16).reshape(4, 128, C).transpose(1, 0, 2)),
        "wdw": np.ascontiguousarray(Wdw.reshape(C, 9)[128:].astype(f32)),
        "wdiag": np.ascontiguousarray(
            np.stack([np.diag(Wdw.reshape(C, 9)[:128, i]) for i in range(9)],
                     axis=1).astype(bf16)),
        "boutT": np.ascontiguousarray(bout.astype(bf16)[None, :]),
        "bdiag": np.kron(np.eye(2, dtype=f32), np.ones((64, 64), f32)).astype(bf16),
    }
    in_maps = []
    for b in range(B):
        fpa = np.pad(fmap[b], [(0, 0), (1, 1), (1, 1)]).astype(bf16)
        fpb = np.pad(fmap[b], [(0, 0), (1, 1), (2, 0)]).astype(bf16)
        m = dict(shared)
        m["fpa"] = ctile(fpa.reshape(C, NPAD))
        m["fpb"] = ctile(fpb.reshape(C, NPAD))
        in_maps.append(m)
    return in_maps


def kernel(fmap, Wq, Wdw, Wkv, Wout, bout, _trace=False, _tmpdir=None,
           _debug=False):
    from concourse.bass_utils import run_bass_kernel_spmd

    fmap, Wq, Wdw, Wkv, Wout, bout = (
        np.asarray(a, np.float32) for a in (fmap, Wq, Wdw, Wkv, Wout, bout))

    key = "nc_dbg" if _debug else "nc"
    if key not in _CACHE:
        _CACHE[key] = _build(debug=_debug)
    nc = _CACHE[key]

    in_maps = _prep_inputs(fmap, Wq, Wdw, Wkv, Wout, bout)
    res = run_bass_kernel_spmd(
        nc, in_maps, core_ids=list(range(B)), trace=_trace, tmpdir=_tmpdir)
    _CACHE["last_result"] = res
    out = np.stack([r["out"] for r in res.results])        # [B, 2, 128, N]
    return out.reshape(B, C, H, W).astype(np.float32)


# revision 10
# speedup vs baseline: 2.1945x; 1.0395x over previous
"""ConvAttention (linear attention with conv projections) on 8 trn2 cores.

Sharding: data-parallel over batch B=8, one image per NeuronCore.

v2 pipeline (channel-major activations [chan, tok], tok = y*64+x):
  qproj   PE -> exp (ACT, [128,1024] tiles) -> expq sbuf
  dw      depthwise3x3: ct0 on GPSIMD (9 STT), ct1 on DVE (TS 4x + TT-add 2x),
          chunked by 16 y-rows to overlap with the kv chain
  kv      PE token-major psum [128,1024]; exp_k (ACT) -> ring; v copy -> ring
  ctx     PE [128,129] blocks: v augmented with ones column so col 128
          accumulates Sk per partition (no separate row-sum / transpose)
  ctxn    DVE scale by 1/Sk * scale into block-diag tile
  Sq      PE bdiag matmul -> psum; 1/Sq on ACT Reciprocal LUT (batched after
          all exps: one table swap) -> rb sbuf
  eqn     DVE expq * rb
  att     PE ctxn^T @ eqn
  gelu    ACT psum -> bf16 sbuf (reuses expq storage)
  out     PE Wout @ g + bias via K=1 ones matmul; DMA psum -> DRAM f32
"""

import numpy as np
import ml_dtypes

B, C, H, W = 8, 256, 64, 64
HEADS, HID = 8, 64
TMP = HEADS * HID            # 512
N = H * W                    # 4096
PAD = 66                     # 64 + 2 halo
NPAD = PAD * PAD             # 4356
NT = 32                      # token tiles of 128
NCHUNK = 4                   # dw/kv chunks of 16 y-rows (1024 tokens)
RING = 4                     # expk / vsb ring depth (token tiles)
SCALE = float(HID) ** -0.5

_CACHE = {}


def _build(debug=False):
    from contextlib import ExitStack

    import concourse.bass as bass
    import concourse.mybir as mybir
    import concourse.tile as tile
    from concourse import bacc

    dt = mybir.dt
    f32, bf16 = dt.float32, dt.bfloat16
    Al = mybir.AluOpType
    Act = mybir.ActivationFunctionType

    nc = bacc.Bacc(
        "TRN2", target_bir_lowering=False, debug=False, enable_asserts=False
    )

    din = {}
    for name, shape, d in [
        ("fpa", [128, 2, NPAD], bf16),       # pad(1,1): x data at cols 1..64
        ("fpb", [128, 2, NPAD], bf16),       # pad(2,0): x data at cols 2..65
        ("wq", [128, 2, TMP], bf16),         # Wq^T   [c, o]
        ("wkv", [128, 2, 2 * TMP], bf16),    # Wkv^T  [c, o]
        ("wout", [128, 4, C], bf16),         # Wout^T [o, c]
        ("wdw", [128, 9], f32),              # depthwise taps, ct1 channels
        ("wdiag", [128, 9, 128], bf16),      # diag(tap) matrices, ct0 chans
        ("bout2", [128, 2], f32),            # bias, c-tiled columns
        ("bdiag", [128, 128], bf16),         # [[J,0],[0,J]] 64x64 ones blocks
    ]:
        din[name] = nc.dram_tensor(name, shape, d, kind="ExternalInput").ap()
    out_d = nc.dram_tensor("out", [2, 128, N], f32, kind="ExternalOutput").ap()
    dbg = {}
    if debug:
        for name, shape, d in [
            ("d_dw", [128, 2, N], bf16),
            ("d_expq", [128, 4, N], bf16),
            ("d_rsk", [128, 4], f32),
            ("d_ctxn", [128, 4, 128], bf16),
            ("d_rb", [128, 4, N], bf16),
        ]:
            dbg[name] = nc.dram_tensor(
                name, shape, d, kind="ExternalOutput").ap()

    with tile.TileContext(nc) as tc, ExitStack() as ctx:
        wp = ctx.enter_context(tc.tile_pool(name="wp", bufs=1))
        sb = ctx.enter_context(tc.tile_pool(name="sb", bufs=1))

        # ---- constants / weights -------------------------------------------
        wq = wp.tile([128, 2, TMP], bf16)
        wkv = wp.tile([128, 2, 2 * TMP], bf16)
        wout = wp.tile([128, 4, C], bf16)
        wdw = wp.tile([128, 9], f32)
        wdiag = wp.tile([128, 9, 128], bf16)
        bout2 = wp.tile([128, 2], f32)
        bdiag = wp.tile([128, 128], bf16)
        # input images in 4 row-bands so early chunks start sooner; the
        # first band and the weights feeding the first matmuls go first
        fpa = sb.tile([128, 2, NPAD], bf16)
        fpb = sb.tile([128, 2, NPAD], bf16)
        bands = [(0, 18), (18, 34), (34, 50), (50, 66)]

        def band_dma(r0, r1):
            sl = slice(r0 * PAD, r1 * PAD)
            nc.sync.dma_start(out=fpa[:, :, sl], in_=din["fpa"][:, :, sl])
            nc.sync.dma_start(out=fpb[:, :, sl], in_=din["fpb"][:, :, sl])

        band_dma(*bands[0])
        for t, name in [(wq, "wq"), (wdw, "wdw"), (wdiag, "wdiag")]:
            nc.sync.dma_start(out=t, in_=din[name])
        band_dma(*bands[1])
        for t, name in [(wkv, "wkv"), (wout, "wout"),
                        (bout2, "bout2"), (bdiag, "bdiag")]:
            nc.sync.dma_start(out=t, in_=din[name])
        for b in bands[2:]:
            band_dma(*b)

        # ---- big sbuf tensors ----------------------------------------------
        dw = sb.tile([128, 2, N], bf16)       # depthwise output, channel-major
        tmpv = sb.tile([128, 1024], bf16)     # DVE tap staging
        expq = sb.tile([128, 4, N], bf16)     # exp(q); later reused as g
        expk = sb.tile([128, RING, 512], bf16)   # token-major ring
        vsb = sb.tile([128, RING, 4, 132], bf16)  # v ring + ones col at 128
        ctxn = sb.tile([128, 4, 128], bf16)   # block-diag scaled ctx per pair
        rsk = sb.tile([128, 4], f32)
        rb = sb.tile([128, 4, N], bf16)       # 1/Sq broadcast per head pair

        nc.gpsimd.memset(vsb[:, :, :, 128:129], 1.0)
        nc.gpsimd.memset(ctxn, 0.0)

        def fview(ct, dy, dx, y0, ny):
            # padded image view [128, ny, 64] for tap (dy, dx), rows y0..y0+ny
            x0 = 1 + dx if dx != 0 else 2
            src = fpa if dx != 0 else fpb
            im = src[:, ct].rearrange("p (y x) -> p y x", y=PAD)
            return im[:, 1 + dy + y0:1 + dy + y0 + ny, x0:x0 + 64]

        def qview(ct, y0, ny):
            im = fpa[:, ct].rearrange("p (y x) -> p y x", y=PAD)
            return im[:, 1 + y0:1 + y0 + ny, 1:65]

        ctxA = ctx.enter_context(ExitStack())
        pa = ctxA.enter_context(
            tc.tile_pool(name="pa", bufs=2, space="PSUM"))
        phC = ctxA.enter_context(
            tc.tile_pool(name="phC", bufs=2, space="PSUM"))
        ctxt = [phC.tile([128, 2, 129], f32, tag="ctx", name=f"ctxt{i}")
                for i in range(2)]

        taps = [(dy, dx) for dy in (-1, 0, 1) for dx in (-1, 0, 1)]

        # ---- q projection + exp (channel-major), chunk-inner ---------------
        for ot in range(4):
            osl = slice(ot * 128, (ot + 1) * 128)
            for ch in range(NCHUNK):
                ps = pa.tile([128, 1024], f32, tag="pa")
                for hf in range(2):
                    y0 = ch * 16 + hf * 8
                    for ct in range(2):
                        nc.tensor.matmul(
                            ps[:, hf * 512:(hf + 1) * 512],
                            wq[:, ct, osl], qview(ct, y0, 8),
                            start=(ct == 0), stop=(ct == 1))
                nc.scalar.activation(
                    expq[:, ot, ch * 1024:(ch + 1) * 1024], ps, Act.Exp)

        # ---- depthwise + kv + ctx, pipelined chunk emission ----------------
        def emit_dw(ch):
            y0 = ch * 16
            csl = slice(ch * 1024, (ch + 1) * 1024)
            # ct0 on PE: 9 diag-matmul taps accumulated in psum, ACT copy out
            dwp = pa.tile([128, 1024], f32, tag="pa")
            for i, (dy, dx) in enumerate(taps):
                for hf in range(2):
                    nc.tensor.matmul(
                        dwp[:, hf * 512:(hf + 1) * 512], wdiag[:, i],
                        fview(0, dy, dx, y0 + hf * 8, 8),
                        start=(i == 0), stop=(i == 8))
            nc.scalar.copy(dw[:, 0, csl], dwp)
            # ct1 on DVE: tensor_scalar 4x + tensor_tensor add 2x
            dwv = dw[:, 1, csl].rearrange("p (y x) -> p y x", y=16)
            tmp3 = tmpv.rearrange("p (y x) -> p y x", y=16)
            for i, (dy, dx) in enumerate(taps):
                fv = fview(1, dy, dx, y0, 16)
                if i == 0:
                    nc.vector.tensor_scalar_mul(dwv, fv, wdw[:, 0:1])
                else:
                    nc.vector.tensor_scalar_mul(tmp3, fv, wdw[:, i:i + 1])
                    nc.vector.tensor_add(dwv, dwv, tmp3)

        def emit_kv(ch):
            for tt in range(ch * 8, ch * 8 + 8):
                tsl = slice(tt * 128, (tt + 1) * 128)
                r = tt % RING
                ps = pa.tile([128, 1024], f32, tag="pa")
                for ct in range(2):
                    nc.tensor.matmul(
                        ps[:, 0:512], dw[:, ct, tsl], wkv[:, ct, 0:512],
                        start=(ct == 0), stop=(ct == 1))
                    nc.tensor.matmul(
                        ps[:, 512:1024], dw[:, ct, tsl], wkv[:, ct, 512:1024],
                        start=(ct == 0), stop=(ct == 1))
                nc.scalar.activation(expk[:, r], ps[:, 0:512], Act.Exp)
                vdst = vsb[:, r, :, 0:128]
                vsrc = ps[:, 512:1024].rearrange("p (a b) -> p a b", a=4)
                nc.scalar.copy(vdst, vsrc)
                for pr in range(4):
                    psl = slice(pr * 128, (pr + 1) * 128)
                    # start=True zeroes the whole 2KB psum bank; only the
                    # first region per bank may use it (pr%2==1 accumulates
                    # onto the bank just zeroed by its pr%2==0 sibling).
                    nc.tensor.matmul(
                        ctxt[pr // 2][:, pr % 2], expk[:, r, psl],
                        vsb[:, r, pr, 0:129],
                        start=(tt == 0 and pr % 2 == 0),
                        stop=(tt == NT - 1),
                        skip_group_check=True)

        phS = ctxA.enter_context(
            tc.tile_pool(name="phS", bufs=2, space="PSUM"))
        rbp32 = ctx.enter_context(tc.tile_pool(name="rbp32", bufs=2))

        def emit_b1(ot):
            # Sq via bdiag matmul; 1/Sq approx + bf16 cast on DVE
            rb32 = rbp32.tile([128, N], f32, tag="rb32")
            for j in range(8):
                base = j * 512
                sqt = phS.tile([128, 512], f32, tag="sq")
                nc.tensor.matmul(
                    sqt, bdiag, expq[:, ot, base:base + 512],
                    start=True, stop=True)
                nc.vector.reciprocal_approx_fast(
                    out=rb32[:, base:base + 512], in_=sqt)
                nc.vector.tensor_copy(
                    rb[:, ot, base:base + 512], rb32[:, base:base + 512])

        emit_dw(0)
        emit_dw(1)
        emit_kv(0)
        emit_b1(0)
        emit_dw(2)
        emit_kv(1)
        emit_b1(1)
        emit_dw(3)
        emit_kv(2)
        emit_b1(2)
        emit_kv(3)
        emit_b1(3)

        # ---- Sk reciprocal + ctxn block-diag build -------------------------
        for pr in range(4):
            nc.vector.reciprocal(
                rsk[:, pr:pr + 1], ctxt[pr // 2][:, pr % 2, 128:129])
        for pr in range(4):
            for hh in range(2):
                rs = slice(hh * 64, (hh + 1) * 64)
                nc.vector.tensor_scalar(
                    out=ctxn[rs, pr, hh * 64:hh * 64 + 64],
                    in0=ctxt[pr // 2][rs, pr % 2, hh * 64:hh * 64 + 64],
                    scalar1=rsk[rs, pr:pr + 1], scalar2=SCALE,
                    op0=Al.mult, op1=Al.mult)
        if debug:
            nc.sync.dma_start(out=dbg["d_dw"], in_=dw)
            nc.sync.dma_start(out=dbg["d_expq"], in_=expq)
            nc.sync.dma_start(out=dbg["d_rsk"], in_=rsk)
            nc.sync.dma_start(out=dbg["d_ctxn"], in_=ctxn)

        ctxA.close()  # release pa + phC + phS
        if debug:
            nc.sync.dma_start(out=dbg["d_rb"], in_=rb)

        # ---- phase B2: eqn, att, gelu, out projection ----------------------
        eqp = ctx.enter_context(tc.tile_pool(name="eqp", bufs=12))
        attp = ctx.enter_context(
            tc.tile_pool(name="attp", bufs=3, space="PSUM"))
        outp = ctx.enter_context(
            tc.tile_pool(name="outp", bufs=2, space="PSUM"))
        osb = ctx.enter_context(tc.tile_pool(name="osb", bufs=3))

        def emit_att(c):
            csl = slice(c * 512, (c + 1) * 512)
            for ot in range(4):
                eqn = eqp.tile([128, 512], bf16, tag="eqn")
                nc.vector.tensor_mul(eqn, expq[:, ot, csl], rb[:, ot, csl])
                att = attp.tile([128, 512], f32, tag="att")
                nc.tensor.matmul(att, ctxn[:, ot], eqn, start=True, stop=True)
                nc.scalar.activation(expq[:, ot, csl], att, Act.Gelu)

        def emit_out(c):
            csl = slice(c * 512, (c + 1) * 512)
            for ct in range(2):
                op = outp.tile([128, 512], f32, tag="op")
                for ot in range(4):
                    nc.tensor.matmul(
                        op, wout[:, ot, ct * 128:(ct + 1) * 128],
                        expq[:, ot, csl], start=(ot == 0), stop=(ot == 3))
                ot_sb = osb.tile([128, 512], f32, tag="osb")
                nc.vector.tensor_scalar_add(ot_sb, op, bout2[:, ct:ct + 1])
                nc.sync.dma_start(out=out_d[ct, :, csl], in_=ot_sb)

        emit_att(0)
        for c in range(1, 8):
            emit_att(c)
            emit_out(c - 1)
        emit_out(7)

    nc.compile()
    return nc


def _prep_inputs(fmap, Wq, Wdw, Wkv, Wout, bout):
    bf16 = ml_dtypes.bfloat16
    f32 = np.float32

    def ctile(a):  # [256, X] -> [128, 2, X]
        return np.ascontiguousarray(
            a.reshape(2, 128, *a.shape[1:]).transpose(1, 0, *range(2, a.ndim + 1)))

    shared = {
        "wq": ctile(Wq.T.astype(bf16)),
        "wkv": ctile(Wkv.T.astype(bf16)),
        "wout": np.ascontiguousarray(
            Wout.T.astype(bf16).reshape(4, 128, C).transpose(1, 0, 2)),
        "wdw": np.ascontiguousarray(Wdw.reshape(C, 9)[128:].astype(f32)),
        "wdiag": np.ascontiguousarray(
            np.stack([np.diag(Wdw.reshape(C, 9)[:128, i]) for i in range(9)],
                     axis=1).astype(bf16)),
        "bout2": np.ascontiguousarray(bout.astype(f32).reshape(2, 128).T),
        "bdiag": np.kron(np.eye(2, dtype=f32), np.ones((64, 64), f32)).astype(bf16),
    }
    in_maps = []
    for b in range(B):
        fpa = np.pad(fmap[b], [(0, 0), (1, 1), (1, 1)]).astype(bf16)
        fpb = np.pad(fmap[b], [(0, 0), (1, 1), (2, 0)]).astype(bf16)
        m = dict(shared)
        m["fpa"] = ctile(fpa.reshape(C, NPAD))
        m["fpb"] = ctile(fpb.reshape(C, NPAD))
        in_maps.append(m)
    return in_maps


def kernel(fmap, Wq, Wdw, Wkv, Wout, bout, _trace=False, _tmpdir=None,
           _debug=False):
    from concourse.bass_utils import run_bass_kernel_spmd

    fmap, Wq, Wdw, Wkv, Wout, bout = (
        np.asarray(a, np.float32) for a in (fmap, Wq, Wdw, Wkv, Wout, bout))

    key = "nc_dbg" if _debug else "nc"
    if key not in _CACHE:
        _CACHE[key] = _build(debug=_debug)
    nc = _CACHE[key]

    in_maps = _prep_inputs(fmap, Wq, Wdw, Wkv, Wout, bout)
    res = run_bass_kernel_spmd(
        nc, in_maps, core_ids=list(range(B)), trace=_trace, tmpdir=_tmpdir)
    _CACHE["last_result"] = res
    out = np.stack([r["out"] for r in res.results])        # [B, 2, 128, N]
    return out.reshape(B, C, H, W).astype(np.float32)


# revision 11
# speedup vs baseline: 2.2457x; 1.0233x over previous
"""ConvAttention (linear attention with conv projections) on 8 trn2 cores.

Sharding: data-parallel over batch B=8, one image per NeuronCore.

v2 pipeline (channel-major activations [chan, tok], tok = y*64+x):
  qproj   PE -> exp (ACT, [128,1024] tiles) -> expq sbuf
  dw      depthwise3x3: ct0 on GPSIMD (9 STT), ct1 on DVE (TS 4x + TT-add 2x),
          chunked by 16 y-rows to overlap with the kv chain
  kv      PE token-major psum [128,1024]; exp_k (ACT) -> ring; v copy -> ring
  ctx     PE [128,129] blocks: v augmented with ones column so col 128
          accumulates Sk per partition (no separate row-sum / transpose)
  ctxn    DVE scale by 1/Sk * scale into block-diag tile
  Sq      PE bdiag matmul -> psum; 1/Sq on ACT Reciprocal LUT (batched after
          all exps: one table swap) -> rb sbuf
  eqn     DVE expq * rb
  att     PE ctxn^T @ eqn
  gelu    ACT psum -> bf16 sbuf (reuses expq storage)
  out     PE Wout @ g + bias via K=1 ones matmul; DMA psum -> DRAM f32
"""

import numpy as np
import ml_dtypes

B, C, H, W = 8, 256, 64, 64
HEADS, HID = 8, 64
TMP = HEADS * HID            # 512
N = H * W                    # 4096
PAD = 66                     # 64 + 2 halo
NPAD = PAD * PAD             # 4356
NT = 32                      # token tiles of 128
NCHUNK = 4                   # dw/kv chunks of 16 y-rows (1024 tokens)
RING = 4                     # expk / vsb ring depth (token tiles)
SCALE = float(HID) ** -0.5

_CACHE = {}


def _build(debug=False):
    from contextlib import ExitStack

    import concourse.bass as bass
    import concourse.mybir as mybir
    import concourse.tile as tile
    from concourse import bacc

    dt = mybir.dt
    f32, bf16 = dt.float32, dt.bfloat16
    Al = mybir.AluOpType
    Act = mybir.ActivationFunctionType

    nc = bacc.Bacc(
        "TRN2", target_bir_lowering=False, debug=False, enable_asserts=False
    )

    din = {}
    for name, shape, d in [
        ("fpa", [128, 2, NPAD], bf16),       # pad(1,1): x data at cols 1..64
        ("fpb", [128, 2, NPAD], bf16),       # pad(2,0): x data at cols 2..65
        ("wq", [128, 2, TMP], bf16),         # Wq^T   [c, o]
        ("wkv", [128, 2, 2 * TMP], bf16),    # Wkv^T  [c, o]
        ("wout", [128, 4, C], bf16),         # Wout^T [o, c]
        ("wdw", [128, 9], f32),              # depthwise taps, ct1 channels
        ("wdiag", [128, 9, 128], bf16),      # diag(tap) matrices, ct0 chans
        ("bout2", [128, 2], f32),            # bias, c-tiled columns
        ("bdiag", [128, 128], bf16),         # [[J,0],[0,J]] 64x64 ones blocks
    ]:
        din[name] = nc.dram_tensor(name, shape, d, kind="ExternalInput").ap()
    out_d = nc.dram_tensor("out", [2, 128, N], f32, kind="ExternalOutput").ap()
    dbg = {}
    if debug:
        for name, shape, d in [
            ("d_dw", [128, 2, N], bf16),
            ("d_expq", [128, 4, N], bf16),
            ("d_rsk", [128, 4], f32),
            ("d_ctxn", [128, 4, 128], bf16),
            ("d_rb", [128, 4, N], bf16),
        ]:
            dbg[name] = nc.dram_tensor(
                name, shape, d, kind="ExternalOutput").ap()

    with tile.TileContext(nc) as tc, ExitStack() as ctx:
        wp = ctx.enter_context(tc.tile_pool(name="wp", bufs=1))
        sb = ctx.enter_context(tc.tile_pool(name="sb", bufs=1))

        # ---- constants / weights -------------------------------------------
        wq = wp.tile([128, 2, TMP], bf16)
        wkv = wp.tile([128, 2, 2 * TMP], bf16)
        wout = wp.tile([128, 4, C], bf16)
        wdw = wp.tile([128, 9], f32)
        wdiag = wp.tile([128, 9, 128], bf16)
        bout2 = wp.tile([128, 2], f32)
        bdiag = wp.tile([128, 128], bf16)
        # input images in 4 row-bands so early chunks start sooner; the
        # first band and the weights feeding the first matmuls go first
        fpa = sb.tile([128, 2, NPAD], bf16)
        fpb = sb.tile([128, 2, NPAD], bf16)
        bands = [(0, 18), (18, 34), (34, 50), (50, 66)]

        def band_dma(r0, r1):
            sl = slice(r0 * PAD, r1 * PAD)
            nc.sync.dma_start(out=fpa[:, :, sl], in_=din["fpa"][:, :, sl])
            nc.sync.dma_start(out=fpb[:, :, sl], in_=din["fpb"][:, :, sl])

        band_dma(*bands[0])
        for t, name in [(wq, "wq"), (wdw, "wdw"), (wdiag, "wdiag")]:
            nc.sync.dma_start(out=t, in_=din[name])
        band_dma(*bands[1])
        for t, name in [(wkv, "wkv"), (wout, "wout"),
                        (bout2, "bout2"), (bdiag, "bdiag")]:
            nc.sync.dma_start(out=t, in_=din[name])
        for b in bands[2:]:
            band_dma(*b)

        # ---- big sbuf tensors ----------------------------------------------
        dw = sb.tile([128, 2, N], bf16)       # depthwise output, channel-major
        tmpv = sb.tile([128, 1024], bf16)     # DVE tap staging
        expq = sb.tile([128, 4, N], bf16)     # exp(q); later reused as g
        expk = sb.tile([128, RING, 512], bf16)   # token-major ring
        vsb = sb.tile([128, RING, 4, 132], bf16)  # v ring + ones col at 128
        ctxn = sb.tile([128, 4, 128], bf16)   # block-diag scaled ctx per pair
        rsk = sb.tile([128, 4], f32)
        rb = sb.tile([128, 4, N], bf16)       # 1/Sq broadcast per head pair

        nc.gpsimd.memset(vsb[:, :, :, 128:129], 1.0)
        nc.gpsimd.memset(ctxn, 0.0)

        def fview(ct, dy, dx, y0, ny):
            # padded image view [128, ny, 64] for tap (dy, dx), rows y0..y0+ny
            x0 = 1 + dx if dx != 0 else 2
            src = fpa if dx != 0 else fpb
            im = src[:, ct].rearrange("p (y x) -> p y x", y=PAD)
            return im[:, 1 + dy + y0:1 + dy + y0 + ny, x0:x0 + 64]

        def qview(ct, y0, ny):
            im = fpa[:, ct].rearrange("p (y x) -> p y x", y=PAD)
            return im[:, 1 + y0:1 + y0 + ny, 1:65]

        ctxA = ctx.enter_context(ExitStack())
        pa = ctxA.enter_context(
            tc.tile_pool(name="pa", bufs=2, space="PSUM"))
        phC = ctxA.enter_context(
            tc.tile_pool(name="phC", bufs=2, space="PSUM"))
        ctxt = [phC.tile([128, 2, 129], f32, tag="ctx", name=f"ctxt{i}")
                for i in range(2)]

        taps = [(dy, dx) for dy in (-1, 0, 1) for dx in (-1, 0, 1)]

        # ---- PE warmup during DMA lead-in (HAM un-throttle) ----------------
        wps = pa.tile([128, 1024], f32, tag="pa")
        for i in range(16):
            nc.tensor.matmul(
                wps[:, 0:128], wq[:, 0, 0:128], wq[:, 1, 0:128],
                start=(i == 0), stop=(i == 15), skip_group_check=True)

        # ---- q projection + exp (channel-major) ----------------------------
        def emit_qp(ot):
            osl = slice(ot * 128, (ot + 1) * 128)
            for ch in range(NCHUNK):
                ps = pa.tile([128, 1024], f32, tag="pa")
                for hf in range(2):
                    y0 = ch * 16 + hf * 8
                    for ct in range(2):
                        nc.tensor.matmul(
                            ps[:, hf * 512:(hf + 1) * 512],
                            wq[:, ct, osl], qview(ct, y0, 8),
                            start=(ct == 0), stop=(ct == 1))
                nc.scalar.activation(
                    expq[:, ot, ch * 1024:(ch + 1) * 1024], ps, Act.Exp)

        # ---- depthwise + kv + ctx, pipelined chunk emission ----------------
        def emit_dw(ch):
            y0 = ch * 16
            csl = slice(ch * 1024, (ch + 1) * 1024)
            # ct0 on PE: 9 diag-matmul taps accumulated in psum, ACT copy out
            dwp = pa.tile([128, 1024], f32, tag="pa")
            for i, (dy, dx) in enumerate(taps):
                for hf in range(2):
                    nc.tensor.matmul(
                        dwp[:, hf * 512:(hf + 1) * 512], wdiag[:, i],
                        fview(0, dy, dx, y0 + hf * 8, 8),
                        start=(i == 0), stop=(i == 8))
            nc.scalar.copy(dw[:, 0, csl], dwp)
            # ct1 on DVE: tensor_scalar 4x + tensor_tensor add 2x
            dwv = dw[:, 1, csl].rearrange("p (y x) -> p y x", y=16)
            tmp3 = tmpv.rearrange("p (y x) -> p y x", y=16)
            for i, (dy, dx) in enumerate(taps):
                fv = fview(1, dy, dx, y0, 16)
                if i == 0:
                    nc.vector.tensor_scalar_mul(dwv, fv, wdw[:, 0:1])
                else:
                    nc.vector.tensor_scalar_mul(tmp3, fv, wdw[:, i:i + 1])
                    nc.vector.tensor_add(dwv, dwv, tmp3)

        def emit_kv(ch):
            for tt in range(ch * 8, ch * 8 + 8):
                tsl = slice(tt * 128, (tt + 1) * 128)
                r = tt % RING
                ps = pa.tile([128, 1024], f32, tag="pa")
                for ct in range(2):
                    nc.tensor.matmul(
                        ps[:, 0:512], dw[:, ct, tsl], wkv[:, ct, 0:512],
                        start=(ct == 0), stop=(ct == 1))
                    nc.tensor.matmul(
                        ps[:, 512:1024], dw[:, ct, tsl], wkv[:, ct, 512:1024],
                        start=(ct == 0), stop=(ct == 1))
                nc.scalar.activation(expk[:, r], ps[:, 0:512], Act.Exp)
                vdst = vsb[:, r, :, 0:128]
                vsrc = ps[:, 512:1024].rearrange("p (a b) -> p a b", a=4)
                nc.scalar.copy(vdst, vsrc)
                for pr in range(4):
                    psl = slice(pr * 128, (pr + 1) * 128)
                    # start=True zeroes the whole 2KB psum bank; only the
                    # first region per bank may use it (pr%2==1 accumulates
                    # onto the bank just zeroed by its pr%2==0 sibling).
                    nc.tensor.matmul(
                        ctxt[pr // 2][:, pr % 2], expk[:, r, psl],
                        vsb[:, r, pr, 0:129],
                        start=(tt == 0 and pr % 2 == 0),
                        stop=(tt == NT - 1),
                        skip_group_check=True)

        phS = ctxA.enter_context(
            tc.tile_pool(name="phS", bufs=2, space="PSUM"))
        rbp32 = ctx.enter_context(tc.tile_pool(name="rbp32", bufs=2))

        def emit_b1(ot):
            # Sq via bdiag matmul; 1/Sq approx + bf16 cast on DVE
            rb32 = rbp32.tile([128, N], f32, tag="rb32")
            for j in range(8):
                base = j * 512
                sqt = phS.tile([128, 512], f32, tag="sq")
                nc.tensor.matmul(
                    sqt, bdiag, expq[:, ot, base:base + 512],
                    start=True, stop=True)
                nc.vector.reciprocal_approx_fast(
                    out=rb32[:, base:base + 512], in_=sqt)
                nc.vector.tensor_copy(
                    rb[:, ot, base:base + 512], rb32[:, base:base + 512])

        emit_dw(0)
        emit_dw(1)
        emit_kv(0)
        emit_qp(0)
        emit_b1(0)
        emit_dw(2)
        emit_kv(1)
        emit_qp(1)
        emit_b1(1)
        emit_dw(3)
        emit_kv(2)
        emit_qp(2)
        emit_b1(2)
        emit_kv(3)
        emit_qp(3)
        emit_b1(3)

        # ---- Sk reciprocal + ctxn block-diag build -------------------------
        for pr in range(4):
            nc.vector.reciprocal(
                rsk[:, pr:pr + 1], ctxt[pr // 2][:, pr % 2, 128:129])
        for pr in range(4):
            for hh in range(2):
                rs = slice(hh * 64, (hh + 1) * 64)
                nc.vector.tensor_scalar(
                    out=ctxn[rs, pr, hh * 64:hh * 64 + 64],
                    in0=ctxt[pr // 2][rs, pr % 2, hh * 64:hh * 64 + 64],
                    scalar1=rsk[rs, pr:pr + 1], scalar2=SCALE,
                    op0=Al.mult, op1=Al.mult)
        if debug:
            nc.sync.dma_start(out=dbg["d_dw"], in_=dw)
            nc.sync.dma_start(out=dbg["d_expq"], in_=expq)
            nc.sync.dma_start(out=dbg["d_rsk"], in_=rsk)
            nc.sync.dma_start(out=dbg["d_ctxn"], in_=ctxn)

        ctxA.close()  # release pa + phC + phS
        if debug:
            nc.sync.dma_start(out=dbg["d_rb"], in_=rb)

        # ---- phase B2: eqn, att, gelu, out projection ----------------------
        eqp = ctx.enter_context(tc.tile_pool(name="eqp", bufs=12))
        attp = ctx.enter_context(
            tc.tile_pool(name="attp", bufs=3, space="PSUM"))
        outp = ctx.enter_context(
            tc.tile_pool(name="outp", bufs=2, space="PSUM"))
        osb = ctx.enter_context(tc.tile_pool(name="osb", bufs=3))

        def emit_att(c):
            csl = slice(c * 512, (c + 1) * 512)
            for ot in range(4):
                eqn = eqp.tile([128, 512], bf16, tag="eqn")
                nc.vector.tensor_mul(eqn, expq[:, ot, csl], rb[:, ot, csl])
                att = attp.tile([128, 512], f32, tag="att")
                nc.tensor.matmul(att, ctxn[:, ot], eqn, start=True, stop=True)
                nc.scalar.activation(expq[:, ot, csl], att, Act.Gelu)

        def emit_out(c):
            csl = slice(c * 512, (c + 1) * 512)
            for ct in range(2):
                op = outp.tile([128, 512], f32, tag="op")
                for ot in range(4):
                    nc.tensor.matmul(
                        op, wout[:, ot, ct * 128:(ct + 1) * 128],
                        expq[:, ot, csl], start=(ot == 0), stop=(ot == 3))
                ot_sb = osb.tile([128, 512], f32, tag="osb")
                nc.vector.tensor_scalar_add(ot_sb, op, bout2[:, ct:ct + 1])
                nc.sync.dma_start(out=out_d[ct, :, csl], in_=ot_sb)

        emit_att(0)
        for c in range(1, 8):
            emit_att(c)
            emit_out(c - 1)
        emit_out(7)

    nc.compile()
    return nc


def _prep_inputs(fmap, Wq, Wdw, Wkv, Wout, bout):
    bf16 = ml_dtypes.bfloat16
    f32 = np.float32

    def ctile(a):  # [256, X] -> [128, 2, X]
        return np.ascontiguousarray(
            a.reshape(2, 128, *a.shape[1:]).transpose(1, 0, *range(2, a.ndim + 1)))

    shared = {
        "wq": ctile(Wq.T.astype(bf16)),
        "wkv": ctile(Wkv.T.astype(bf16)),
        "wout": np.ascontiguousarray(
            Wout.T.astype(bf16).reshape(4, 128, C).transpose(1, 0, 2)),
        "wdw": np.ascontiguousarray(Wdw.reshape(C, 9)[128:].astype(f32)),
        "wdiag": np.ascontiguousarray(
            np.stack([np.diag(Wdw.reshape(C, 9)[:128, i]) for i in range(9)],
                     axis=1).astype(bf16)),
        "bout2": np.ascontiguousarray(bout.astype(f32).reshape(2, 128).T),
        "bdiag": np.kron(np.eye(2, dtype=f32), np.ones((64, 64), f32)).astype(bf16),
    }
    in_maps = []
    for b in range(B):
        fpa = np.pad(fmap[b], [(0, 0), (1, 1), (1, 1)]).astype(bf16)
        fpb = np.pad(fmap[b], [(0, 0), (1, 1), (2, 0)]).astype(bf16)
        m = dict(shared)
        m["fpa"] = ctile(fpa.reshape(C, NPAD))
        m["fpb"] = ctile(fpb.reshape(C, NPAD))
        in_maps.append(m)
    return in_maps


def kernel(fmap, Wq, Wdw, Wkv, Wout, bout, _trace=False, _tmpdir=None,
           _debug=False):
    from concourse.bass_utils import run_bass_kernel_spmd

    fmap, Wq, Wdw, Wkv, Wout, bout = (
        np.asarray(a, np.float32) for a in (fmap, Wq, Wdw, Wkv, Wout, bout))

    key = "nc_dbg" if _debug else "nc"
    if key not in _CACHE:
        _CACHE[key] = _build(debug=_debug)
    nc = _CACHE[key]

    in_maps = _prep_inputs(fmap, Wq, Wdw, Wkv, Wout, bout)
    res = run_bass_kernel_spmd(
        nc, in_maps, core_ids=list(range(B)), trace=_trace, tmpdir=_tmpdir)
    _CACHE["last_result"] = res
    out = np.stack([r["out"] for r in res.results])        # [B, 2, 128, N]
    return out.reshape(B, C, H, W).astype(np.float32)
